# revision 47
# baseline (speedup 1.0000x reference)
"""CQAttention (BiDAF context-query attention) Trainium2 kernel, v18.

Shapes: C (32,128,1024), Q (32,128,512), W (32768,1,384) -> out (32,512,1024).
Data-parallel across 8 NeuronCores: 4 batches per core, no collectives.

Device work per batch b (D=128, CL=1024 -> NK=8 c-chunks, QL=512 -> NJ=4):
  stage1: Z chunks = UT_k^T @ Q      (8 MM N=512, bf16)
          E = exp(Z + rbias)         (8 ACT exps, bias port)
          rowsum: ACT accum_out on chunks < SPLIT_ACC, one grouped DVE
          tensor_reduce for the rest (balances ACT vs DVE)
  xbar:   F = E^T (raw, unnormalized) via DMA xbar block-transposes
          (halves; quarters for the last batch to shorten the drain)
  stage2g: G|colsum = E^T @ [C^T|1]  (32 MM N=129), Gn = G/colsum
          (recip + tensor_scalar on DVE)
  stage3: A_raw^T = Qt^T @ F, B_raw^T = Gn^T @ F (16 MM N=512),
          PSUM->SBUF bf16 copies on DVE (ACT in the drain),
          out = [A_raw; B_raw] bf16
Host pre:  UT = wq^T + wqc^T*C (f32), rbias = sum_d wc*C^T, packs
           PK = [Q | Q^T | UT | C^T+ones] bf16.
Host post: S1's row normalization commutes with the A/B matmuls:
           A = A_raw / rowsum[c], B = B_raw / rowsum[c] (columnwise),
           then out = [C, A, C*A, C*B] in f32.

Scheduling notes (hard-won):
  - The exp cadence (~0.9us/chunk on ACT) paces stage1 psum recycling, so
    stage2g(b-1)/stage3(b-1) blocks are interleaved between stage1 matmuls
    as filler; the PE queue then always holds ready work.
  - The tile scheduler serializes dma_start_transpose against EVERY other
    DMA (HW deadlock workaround), so the kernel is structured as
    loads -> all transposes -> all stores, with the stores ordered after
    the final transpose via explicit add_dep_helper edges.  A
    tile_wait_until clock gate is NOT safe for that ordering: the
    scheduler elides semaphores it believes timing covers, and per-core
    clock skew then races the stores against the copies.
  - Emitting a previous batch's leftover stage3 blocks after the next
    batch's first stage1 matmuls reproducibly corrupts a few hundred
    outputs (framework scheduling edge case) -- leftovers must run at the
    end of their own emission window.
"""

import ml_dtypes
import numpy as np

import concourse.bacc as bacc
import concourse.mybir as mybir
from concourse import tile
from concourse.tile import add_dep_helper
from concourse.bass_utils import run_bass_kernel_spmd

B, D, CL, QL = 32, 128, 1024, 512
NCORES = 8
BPC = B // NCORES          # batches per core
NK = CL // D               # 8 c-chunks of 128
NJ = QL // D               # 4 q-chunks of 128

# packed bf16 input column offsets: [Qb | UT | Qt | cto] -- batch 0's
# first load covers exactly what stage1 needs (Qb+UT); Qt/cto follow
O_QB = 0
O_UT = O_QB + QL
O_QT = O_UT + CL
O_CTO = O_QT + NJ * D
PKW = O_CTO + NK * (D + 1)   # 3080

F32 = mybir.dt.float32
BF16 = mybir.dt.bfloat16
EXP = mybir.ActivationFunctionType.Exp
BF = ml_dtypes.bfloat16

_NC = None
RUN_KWARGS = {}        # test harness can set e.g. {"trace": True}
LAST_RESULT = None     # last BassKernelResults (for exec_time_ns / trace)

N_WARM = 5             # HAM warm-up dummy matmuls
SPLIT_ACC = 4          # rowsum chunks 0..SPLIT_ACC-1 via ACT accum, rest DVE


class _Batch:
    """Per-batch tiles and views."""

    def __init__(self, nc, ipool, pool, b, PK_d, RB_d, OUT_d):
        self.b = b
        self.OUT_d = OUT_d[b]
        self.pk = ipool.tile([D, PKW], BF16, tag=f"pk{b}")
        self.rb = ipool.tile([D, NK], F32, tag=f"rb{b}")
        self.Qb = self.pk[:, O_QB:O_QB + QL]
        self.Qt = self.pk[:, O_QT:O_QT + NJ * D]
        self.UT = self.pk[:, O_UT:O_UT + CL]
        cto = self.pk[:, O_CTO:O_CTO + NK * (D + 1)]
        self.cto_v = cto.rearrange("p (k e) -> p k e", k=NK)
        # pooled per-batch working tiles
        self.E = pool.tile([D, NK * QL], BF16, tag="E")
        self.F = pool.tile([D, NJ * CL], BF16, tag="F", bufs=3)
        self.Fr = self.F.rearrange("p (k j c) -> p k j c", k=NK, j=NJ)
        self.Gn = pool.tile([D, NJ * D], BF16, tag="Gn")
        self.crec = pool.tile([D, NJ], F32, tag="crec")
        self.AB = pool.tile([D, 4 * QL], BF16, tag="AB", bufs=4)

    def load(self, nc, split=False):
        if split:
            # batch 0: [Qb|UT chunks 0-1] lands first and gates z(0,0);
            # same-tile loads serialize (tile-granular WAW), so the second
            # piece is sized to land before the exp cadence reaches chunk 2;
            # Qt/cto are only needed by stage3/stage2g one batch later
            c1 = O_UT + 2 * D
            nc.sync.dma_start(self.pk[:, 0:c1], _PK_D[self.b][:, 0:c1])
            nc.sync.dma_start(self.rb[:], _RB_D[self.b])
            nc.sync.dma_start(self.pk[:, c1:O_QT], _PK_D[self.b][:, c1:O_QT])
            nc.sync.dma_start(self.pk[:, O_QT:], _PK_D[self.b][:, O_QT:])
        else:
            nc.sync.dma_start(self.pk[:], _PK_D[self.b])
            nc.sync.dma_start(self.rb[:], _RB_D[self.b])


_PK_D = None
_RB_D = None


def _emit_z(nc, t, psS, rs_all, k):
    # one stage1 chunk: Z matmul + exp (+rowsum accum on early chunks)
    ps = psS.tile([D, QL], F32, tag="ps")
    nc.tensor.matmul(ps[:], t.UT[:, k * D:(k + 1) * D], t.Qb[:],
                     start=True, stop=True)
    kw = {}
    if k < SPLIT_ACC:
        kw["accum_out"] = rs_all[:, t.b * NK + k: t.b * NK + k + 1]
    nc.scalar.activation(t.E[:, k * QL:(k + 1) * QL], ps[:], EXP,
                         bias=t.rb[:, k:k + 1], **kw)


def _emit_reduce(nc, t, rs_all):
    # rowsum for the late chunks, one grouped free-dim reduce
    nr = NK - SPLIT_ACC
    nc.vector.tensor_reduce(
        rs_all[:, t.b * NK + SPLIT_ACC: (t.b + 1) * NK],
        t.E[:, SPLIT_ACC * QL:].rearrange("p (k q) -> p k q", k=nr),
        axis=mybir.AxisListType.X, op=mybir.AluOpType.add)


def _emit_xbar(nc, t, h, nsplit=2):
    # F part h of nsplit = (E part h)^T via DMA xbar block-transpose
    m = NK * NJ // nsplit
    return nc.sync.dma_start_transpose(
        t.F[:, h * m * D:(h + 1) * m * D].rearrange(
            "p (m c) -> p m c", m=m),
        t.E[:, h * m * D:(h + 1) * m * D])


def _emit_g(nc, t, psG, j):
    # one stage2g block: G|colsum column chunk j + Gn scale
    psg = psG.tile([D, D + 1], F32, tag="psg")
    for k in range(NK):
        nc.tensor.matmul(psg[:],
                         t.E[:, k * QL + j * D: k * QL + (j + 1) * D],
                         t.cto_v[:, k, :],
                         start=(k == 0), stop=(k == NK - 1))
    nc.vector.reciprocal(t.crec[:, j:j + 1], psg[:, D:D + 1])
    nc.vector.tensor_scalar_mul(t.Gn[:, j * D:(j + 1) * D], psg[:, 0:D],
                                t.crec[:, j:j + 1])


def _emit_s3(nc, t, psAB, x, on_act=False):
    # one stage3 block: x = 0..3 -> (A,h0), (B,h0), (A,h1), (B,h1)
    h, isb = x // 2, x % 2
    ps = psAB.tile([D, QL], F32, tag="ps")
    for j in range(NJ):
        lhsT = (t.Gn[:, j * D:(j + 1) * D] if isb
                else t.Qt[:, j * D:(j + 1) * D])
        nc.tensor.matmul(ps[:], lhsT, t.Fr[:, NJ * h:NJ * h + NJ, j, :],
                         start=(j == 0), stop=(j == NJ - 1))
    dst = t.AB[:, (2 * isb + h) * QL:(2 * isb + h + 1) * QL]
    if on_act and isb:
        # drain phase: B copies (which gate the final store) go on the
        # now-idle ACT; A copies stay on DVE so neither queue serializes
        # all four
        nc.scalar.copy(dst, ps[:])
    else:
        nc.vector.tensor_copy(dst, ps[:])


def _emit_out(nc, t, after=None):
    # [A_raw; B_raw] bf16, one DMA.  All output stores are ordered AFTER
    # the last xbar transpose via an explicit dependency edge: the tile
    # scheduler serializes dma_start_transpose against every other DMA (HW
    # deadlock workaround), so a store scheduled into the transpose stream
    # stalls the pipeline.  (A tile_wait_until clock gate is NOT safe here:
    # the scheduler elides semaphores it thinks timing covers, and per-core
    # HW skew then races the store against the copies.)
    ins = nc.sync.dma_start(t.OUT_d.rearrange("(r p) c -> p r c", p=D),
                            t.AB.rearrange("p (r c) -> p r c", r=2))
    if after is not None:
        add_dep_helper(ins.ins, after.ins, reason="store after last transpose")
    return ins


def _build():
    global _PK_D, _RB_D
    nc = bacc.Bacc("TRN2", debug=False, num_devices=NCORES)

    PK_d = nc.dram_tensor("PK", [BPC, D, PKW], BF16, kind="ExternalInput").ap()
    RB_d = nc.dram_tensor("RB", [BPC, D, NK], F32, kind="ExternalInput").ap()
    OUT_d = nc.dram_tensor("OUT", [BPC, 2 * D, CL], BF16,
                           kind="ExternalOutput").ap()
    RS_d = nc.dram_tensor("RS", [D, BPC * NK], F32, kind="ExternalOutput").ap()
    _PK_D, _RB_D = PK_d, RB_d

    with tile.TileContext(nc) as tc:
        with (
            tc.tile_pool(name="ins", bufs=1) as ipool,
            tc.tile_pool(name="work", bufs=3) as pool,
            tc.tile_pool(name="psS", bufs=3, space="PSUM") as psS,
            tc.tile_pool(name="psG", bufs=2, space="PSUM") as psG,
            tc.tile_pool(name="psAB", bufs=3, space="PSUM") as psAB,
        ):
            ts = [_Batch(nc, ipool, pool, b, PK_d, RB_d, OUT_d)
                  for b in range(BPC)]
            rs_all = ipool.tile([D, BPC * NK], F32, tag="rs_all")
            dmyL = ipool.tile([D, D], BF16, tag="dmyL")
            dmyR = ipool.tile([D, QL], BF16, tag="dmyR")
            dmyA = ipool.tile([D, 2], F32, tag="dmyA")
            # input loads first: batch 0 ASAP
            ts[0].load(nc, split=True)
            for t in ts[1:]:
                t.load(nc)
            # dmyA memset first: the table-preload dummy exp waits on it,
            # and the 1.3us ACT_TABLE_LOAD must clear before the first real
            # exp needs the ACT engine
            nc.vector.memset(dmyA[:, 0:1], 0.0)
            nc.scalar.activation(dmyA[:, 1:2], dmyA[:, 0:1], EXP)
            nc.vector.memset(dmyL[:], 0.0)
            nc.vector.memset(dmyR[:], 0.0)
            # HAM warm-up: trip the PE activity window to 8/8 (2.4 GHz)
            for _ in range(N_WARM):
                psd = psS.tile([D, QL], F32, tag="ps")
                nc.tensor.matmul(psd[:], dmyL[:], dmyR[:],
                                 start=True, stop=True)

            def fillers(b):
                # filler blocks (for batch b-1) usable while batch b's exps
                # pace the PE.  s3 blocks x: 0=(A,h0) 1=(B,h0) 2=(A,h1)
                # 3=(B,h1); order keeps A blocks early (need only F) and B
                # blocks after all g blocks (need Gn complete).  Consumed
                # via a carried deque (slots k=1..7 each batch) so no
                # leftover block ever sits between batches ahead of the
                # next batch's first z matmul.
                if b < 1:
                    return []
                fb = b - 1
                return [("g", fb, 0), ("g", fb, 1), ("g", fb, 2),
                        ("g", fb, 3), ("s3", fb, 0), ("s3", fb, 2),
                        ("s3", fb, 1), ("s3", fb, 3)]

            def run_filler(f, on_act=False):
                kind, fb, i = f
                if kind == "g":
                    _emit_g(nc, ts[fb], psG, i)
                else:
                    _emit_s3(nc, ts[fb], psAB, i, on_act=on_act)

            last = BPC - 1
            for b in range(BPC):
                t = ts[b]
                fl = fillers(b)
                for k in range(3):
                    _emit_z(nc, t, psS, rs_all, k)
                for k in range(3, NK):
                    if fl:
                        run_filler(fl.pop(0))
                    _emit_z(nc, t, psS, rs_all, k)
                    # last batch: quarter xbars so the drain's F turnaround
                    # after the final exp is ~1.3us instead of ~3us
                    if b == last:
                        if k == 3:
                            _emit_xbar(nc, t, 0, nsplit=4)
                        elif k == 5:
                            _emit_xbar(nc, t, 1, nsplit=4)
                        elif k == 7:
                            _emit_xbar(nc, t, 2, nsplit=4)
                    elif k == 3:
                        _emit_xbar(nc, t, 0)
                if b == last:
                    last_xbar = _emit_xbar(nc, t, 3, nsplit=4)
                else:
                    _emit_xbar(nc, t, 1)
                for f in fl:
                    run_filler(f)
                if b >= 1:
                    # reduce for the PREVIOUS batch: its E is long complete,
                    # so this never blocks the DVE queue head (emitting
                    # reduce(b) here would stall crec/Gn behind it waiting
                    # for exp(b,7), jamming psG recycling and the PE)
                    _emit_reduce(nc, ts[b - 1], rs_all)
            # all output stores AFTER the last xbar (no transpose/store
            # DMA-mode transitions mid-kernel), enforced with explicit
            # dependency edges on the final transpose
            for fb in range(BPC - 1):
                _emit_out(nc, ts[fb], after=last_xbar)
            # drain: A-h0 first (it needs only the first two F quarters,
            # ready before the final exps finish), then g(3) which needs
            # full E(3), then the rest; drain B copies go on ACT
            drain = fillers(BPC)
            drain.insert(0, drain.pop(4))   # Ah0 to the front
            for f in drain:
                run_filler(f, on_act=True)
            _emit_reduce(nc, ts[last], rs_all)
            ins = nc.sync.dma_start(RS_d, rs_all[:])
            add_dep_helper(ins.ins, last_xbar.ins, reason="RS after last transpose")
            # last batch ships as A-half then B-half so the A rows go out
            # while the B drain copies still run
            t3 = ts[last]
            for isb in range(2):
                ins = nc.sync.dma_start(
                    t3.OUT_d[isb * D:(isb + 1) * D, :],
                    t3.AB[:, isb * 2 * QL:(isb + 1) * 2 * QL])
                add_dep_helper(ins.ins, last_xbar.ins,
                               reason="store after last transpose")
    nc.compile()
    return nc


def _get_nc():
    global _NC
    if _NC is None:
        _NC = _build()
    return _NC


def _prep_core(Ci, Qi, Wi):
    """Host-side layout prep for one core's shard (transposes/casts and
    cheap elementwise folds: UT = wq^T + wqc^T*C, rbias = sum_d wc*C^T)."""
    bpc = Ci.shape[0]
    pk = np.empty((bpc, D, PKW), dtype=BF)
    pk[:, :, O_QB:O_QB + QL] = Qi.astype(BF)
    qt = Qi.transpose(0, 2, 1).reshape(bpc, NJ, D, D).transpose(0, 2, 1, 3)
    pk[:, :, O_QT:O_QT + NJ * D] = qt.reshape(bpc, D, NJ * D).astype(BF)
    wqT = Wi[:, :, 0:D].transpose(0, 2, 1)          # (bpc, D, CL) f32
    wqcT = Wi[:, :, 2 * D:3 * D].transpose(0, 2, 1)
    pk[:, :, O_UT:O_UT + CL] = (wqT + wqcT * Ci).astype(BF)
    ct = Ci.transpose(0, 2, 1).reshape(bpc, NK, D, D).transpose(0, 2, 1, 3)
    cto = np.concatenate(
        [ct, np.ones((bpc, D, NK, 1), dtype=np.float32)], axis=3)
    pk[:, :, O_CTO:O_CTO + NK * (D + 1)] = (
        cto.reshape(bpc, D, NK * (D + 1)).astype(BF))
    # rbias[c] = sum_d wc[c,d] * C[d,c], laid out [c_local, k]
    rb = np.einsum('bcd,bdc->bc', Wi[:, :, D:2 * D], Ci).astype(np.float32)
    rb = rb.reshape(bpc, NK, D).transpose(0, 2, 1)  # (bpc, c_local, k)
    return {"PK": pk, "RB": np.ascontiguousarray(rb)}


def kernel(C, Q, W):
    C = np.ascontiguousarray(np.asarray(C, dtype=np.float32))
    Q = np.ascontiguousarray(np.asarray(Q, dtype=np.float32))
    W = np.ascontiguousarray(np.asarray(W, dtype=np.float32)).reshape(B, CL, 3 * D)
    in_maps = [
        _prep_core(C[i * BPC:(i + 1) * BPC],
                   Q[i * BPC:(i + 1) * BPC],
                   W[i * BPC:(i + 1) * BPC])
        for i in range(NCORES)
    ]
    nc = _get_nc()
    res = run_bass_kernel_spmd(nc, in_maps, core_ids=list(range(NCORES)), **RUN_KWARGS)
    global LAST_RESULT
    LAST_RESULT = res
    AB = np.concatenate([res.results[i]["OUT"] for i in range(NCORES)], axis=0)
    RS = np.stack([res.results[i]["RS"] for i in range(NCORES)])  # [8,D,BPC*NK]
    # host post: columnwise 1/rowsum (S1 normalization commuted out of the
    # A/B matmuls), then assemble [C, A, C*A, C*B]
    rowsum = (RS.reshape(NCORES, D, BPC, NK).transpose(0, 2, 3, 1)
              .reshape(B, CL))                       # [b, c] (k-major)
    h = (1.0 / rowsum)[:, None, :]                   # [b, 1, c]
    A = AB[:, 0:D].astype(np.float32) * h
    Bm = AB[:, D:2 * D].astype(np.float32) * h
    out = np.empty((B, 4 * D, CL), dtype=np.float32)
    out[:, 0:D] = C
    out[:, D:2 * D] = A
    out[:, 2 * D:3 * D] = C * A
    out[:, 3 * D:4 * D] = C * Bm
    return out


# revision 48
# speedup vs baseline: 1.0141x; 1.0141x over previous
"""CQAttention (BiDAF context-query attention) Trainium2 kernel, v18.

Shapes: C (32,128,1024), Q (32,128,512), W (32768,1,384) -> out (32,512,1024).
Data-parallel across 8 NeuronCores: 4 batches per core, no collectives.

Device work per batch b (D=128, CL=1024 -> NK=8 c-chunks, QL=512 -> NJ=4):
  stage1: Z chunks = UT_k^T @ Q      (8 MM N=512, bf16)
          E = exp(Z + rbias)         (8 ACT exps, bias port)
          rowsum: ACT accum_out on chunks < SPLIT_ACC, one grouped DVE
          tensor_reduce for the rest (balances ACT vs DVE)
  xbar:   F = E^T (raw, unnormalized) via DMA xbar block-transposes
          (halves; quarters for the last batch to shorten the drain)
  stage2g: G|colsum = E^T @ [C^T|1]  (32 MM N=129), Gn = G/colsum
          (recip + tensor_scalar on DVE)
  stage3: A_raw^T = Qt^T @ F, B_raw^T = Gn^T @ F (16 MM N=512),
          PSUM->SBUF bf16 copies on DVE (ACT in the drain),
          out = [A_raw; B_raw] bf16
Host pre:  UT = wq^T + wqc^T*C (f32), rbias = sum_d wc*C^T, packs
           PK = [Q | Q^T | UT | C^T+ones] bf16.
Host post: S1's row normalization commutes with the A/B matmuls:
           A = A_raw / rowsum[c], B = B_raw / rowsum[c] (columnwise),
           then out = [C, A, C*A, C*B] in f32.

Scheduling notes (hard-won):
  - The exp cadence (~0.9us/chunk on ACT) paces stage1 psum recycling, so
    stage2g(b-1)/stage3(b-1) blocks are interleaved between stage1 matmuls
    as filler; the PE queue then always holds ready work.
  - The tile scheduler serializes dma_start_transpose against EVERY other
    DMA (HW deadlock workaround), so the kernel is structured as
    loads -> all transposes -> all stores, with the stores ordered after
    the final transpose via explicit add_dep_helper edges.  A
    tile_wait_until clock gate is NOT safe for that ordering: the
    scheduler elides semaphores it believes timing covers, and per-core
    clock skew then races the stores against the copies.
  - Emitting a previous batch's leftover stage3 blocks after the next
    batch's first stage1 matmuls reproducibly corrupts a few hundred
    outputs (framework scheduling edge case) -- leftovers must run at the
    end of their own emission window.
"""

import ml_dtypes
import numpy as np

import concourse.bacc as bacc
import concourse.mybir as mybir
from concourse import tile
from concourse.tile import add_dep_helper
from concourse.bass_utils import run_bass_kernel_spmd

B, D, CL, QL = 32, 128, 1024, 512
NCORES = 8
BPC = B // NCORES          # batches per core
NK = CL // D               # 8 c-chunks of 128
NJ = QL // D               # 4 q-chunks of 128

# packed bf16 input column offsets: [Qb | UT | Qt | cto] -- batch 0's
# first load covers exactly what stage1 needs (Qb+UT); Qt/cto follow
O_QB = 0
O_UT = O_QB + QL
O_QT = O_UT + CL
O_CTO = O_QT + NJ * D
PKW = O_CTO + NK * (D + 1)   # 3080

F32 = mybir.dt.float32
BF16 = mybir.dt.bfloat16
EXP = mybir.ActivationFunctionType.Exp
BF = ml_dtypes.bfloat16

_NC = None
RUN_KWARGS = {}        # test harness can set e.g. {"trace": True}
LAST_RESULT = None     # last BassKernelResults (for exec_time_ns / trace)

N_WARM = 5             # HAM warm-up dummy matmuls
SPLIT_ACC = 4          # rowsum chunks 0..SPLIT_ACC-1 via ACT accum, rest DVE


class _Batch:
    """Per-batch tiles and views."""

    def __init__(self, nc, ipool, pool, b, PK_d, RB_d, OUT_d):
        self.b = b
        self.OUT_d = OUT_d[b]
        self.pk = ipool.tile([D, PKW], BF16, tag=f"pk{b}")
        self.rb = ipool.tile([D, NK], F32, tag=f"rb{b}")
        self.Qb = self.pk[:, O_QB:O_QB + QL]
        self.Qt = self.pk[:, O_QT:O_QT + NJ * D]
        self.UT = self.pk[:, O_UT:O_UT + CL]
        cto = self.pk[:, O_CTO:O_CTO + NK * (D + 1)]
        self.cto_v = cto.rearrange("p (k e) -> p k e", k=NK)
        # pooled per-batch working tiles
        self.E = pool.tile([D, NK * QL], BF16, tag="E")
        self.F = pool.tile([D, NJ * CL], BF16, tag="F", bufs=3)
        self.Fr = self.F.rearrange("p (k j c) -> p k j c", k=NK, j=NJ)
        self.Gn = pool.tile([D, NJ * D], BF16, tag="Gn")
        self.crec = pool.tile([D, NJ], F32, tag="crec")
        self.AB = pool.tile([D, 4 * QL], BF16, tag="AB", bufs=4)

    def load(self, nc, split=False):
        if split:
            # batch 0: [Qb|UT chunks 0-1] lands first and gates z(0,0);
            # same-tile loads serialize (tile-granular WAW), so the second
            # piece is sized to land before the exp cadence reaches chunk 2;
            # Qt/cto are only needed by stage3/stage2g one batch later
            c1 = O_UT + 2 * D
            nc.sync.dma_start(self.pk[:, 0:c1], _PK_D[self.b][:, 0:c1])
            nc.sync.dma_start(self.rb[:], _RB_D[self.b])
            nc.sync.dma_start(self.pk[:, c1:O_QT], _PK_D[self.b][:, c1:O_QT])
            nc.sync.dma_start(self.pk[:, O_QT:], _PK_D[self.b][:, O_QT:])
        else:
            nc.sync.dma_start(self.pk[:], _PK_D[self.b])
            nc.sync.dma_start(self.rb[:], _RB_D[self.b])


_PK_D = None
_RB_D = None


def _emit_z(nc, t, psS, rs_all, k):
    # one stage1 chunk: Z matmul + exp (+rowsum accum on early chunks)
    ps = psS.tile([D, QL], F32, tag="ps")
    nc.tensor.matmul(ps[:], t.UT[:, k * D:(k + 1) * D], t.Qb[:],
                     start=True, stop=True)
    kw = {}
    if k < SPLIT_ACC:
        kw["accum_out"] = rs_all[:, t.b * NK + k: t.b * NK + k + 1]
    nc.scalar.activation(t.E[:, k * QL:(k + 1) * QL], ps[:], EXP,
                         bias=t.rb[:, k:k + 1], **kw)


def _emit_reduce(nc, t, rs_all):
    # rowsum for the late chunks, one grouped free-dim reduce
    nr = NK - SPLIT_ACC
    nc.vector.tensor_reduce(
        rs_all[:, t.b * NK + SPLIT_ACC: (t.b + 1) * NK],
        t.E[:, SPLIT_ACC * QL:].rearrange("p (k q) -> p k q", k=nr),
        axis=mybir.AxisListType.X, op=mybir.AluOpType.add)


def _emit_xbar(nc, t, h, nsplit=2):
    # F part h of nsplit = (E part h)^T via DMA xbar block-transpose
    m = NK * NJ // nsplit
    return nc.sync.dma_start_transpose(
        t.F[:, h * m * D:(h + 1) * m * D].rearrange(
            "p (m c) -> p m c", m=m),
        t.E[:, h * m * D:(h + 1) * m * D])


def _emit_g(nc, t, psG, j):
    # one stage2g block: G|colsum column chunk j + Gn scale
    psg = psG.tile([D, D + 1], F32, tag="psg")
    for k in range(NK):
        nc.tensor.matmul(psg[:],
                         t.E[:, k * QL + j * D: k * QL + (j + 1) * D],
                         t.cto_v[:, k, :],
                         start=(k == 0), stop=(k == NK - 1))
    nc.vector.reciprocal(t.crec[:, j:j + 1], psg[:, D:D + 1])
    nc.vector.tensor_scalar_mul(t.Gn[:, j * D:(j + 1) * D], psg[:, 0:D],
                                t.crec[:, j:j + 1])


def _emit_s3(nc, t, psAB, x, on_act=False):
    # one stage3 block: x = 0..3 -> (A,h0), (B,h0), (A,h1), (B,h1)
    h, isb = x // 2, x % 2
    ps = psAB.tile([D, QL], F32, tag="ps")
    for j in range(NJ):
        lhsT = (t.Gn[:, j * D:(j + 1) * D] if isb
                else t.Qt[:, j * D:(j + 1) * D])
        nc.tensor.matmul(ps[:], lhsT, t.Fr[:, NJ * h:NJ * h + NJ, j, :],
                         start=(j == 0), stop=(j == NJ - 1))
    dst = t.AB[:, (2 * isb + h) * QL:(2 * isb + h + 1) * QL]
    if on_act and isb:
        # drain phase: B copies (which gate the final store) go on the
        # now-idle ACT; A copies stay on DVE so neither queue serializes
        # all four
        nc.scalar.copy(dst, ps[:])
    else:
        nc.vector.tensor_copy(dst, ps[:])


def _emit_out(nc, t, after=None):
    # [A_raw; B_raw] bf16, one DMA.  All output stores are ordered AFTER
    # the last xbar transpose via an explicit dependency edge: the tile
    # scheduler serializes dma_start_transpose against every other DMA (HW
    # deadlock workaround), so a store scheduled into the transpose stream
    # stalls the pipeline.  (A tile_wait_until clock gate is NOT safe here:
    # the scheduler elides semaphores it thinks timing covers, and per-core
    # HW skew then races the store against the copies.)
    ins = nc.sync.dma_start(t.OUT_d.rearrange("(r p) c -> p r c", p=D),
                            t.AB.rearrange("p (r c) -> p r c", r=2))
    if after is not None:
        add_dep_helper(ins.ins, after.ins, reason="store after last transpose")
    return ins


def _build():
    global _PK_D, _RB_D
    nc = bacc.Bacc("TRN2", debug=False, num_devices=NCORES)

    PK_d = nc.dram_tensor("PK", [BPC, D, PKW], BF16, kind="ExternalInput").ap()
    RB_d = nc.dram_tensor("RB", [BPC, D, NK], F32, kind="ExternalInput").ap()
    OUT_d = nc.dram_tensor("OUT", [BPC, 2 * D, CL], BF16,
                           kind="ExternalOutput").ap()
    RS_d = nc.dram_tensor("RS", [D, BPC * NK], F32, kind="ExternalOutput").ap()
    _PK_D, _RB_D = PK_d, RB_d

    with tile.TileContext(nc) as tc:
        with (
            tc.tile_pool(name="ins", bufs=1) as ipool,
            tc.tile_pool(name="work", bufs=3) as pool,
            tc.tile_pool(name="psS", bufs=3, space="PSUM") as psS,
            tc.tile_pool(name="psG", bufs=2, space="PSUM") as psG,
            tc.tile_pool(name="psAB", bufs=3, space="PSUM") as psAB,
        ):
            ts = [_Batch(nc, ipool, pool, b, PK_d, RB_d, OUT_d)
                  for b in range(BPC)]
            rs_all = ipool.tile([D, BPC * NK], F32, tag="rs_all")
            dmyL = ipool.tile([D, D], BF16, tag="dmyL")
            dmyR = ipool.tile([D, QL], BF16, tag="dmyR")
            dmyA = ipool.tile([D, 2], F32, tag="dmyA")
            # input loads first: batch 0 ASAP
            ts[0].load(nc, split=True)
            for t in ts[1:]:
                t.load(nc)
            # dmyA memset first: the table-preload dummy exp waits on it,
            # and the 1.3us ACT_TABLE_LOAD must clear before the first real
            # exp needs the ACT engine
            nc.vector.memset(dmyA[:, 0:1], 0.0)
            nc.scalar.activation(dmyA[:, 1:2], dmyA[:, 0:1], EXP)
            nc.vector.memset(dmyL[:], 0.0)
            nc.vector.memset(dmyR[:], 0.0)
            # HAM warm-up: trip the PE activity window to 8/8 (2.4 GHz)
            for _ in range(N_WARM):
                psd = psS.tile([D, QL], F32, tag="ps")
                nc.tensor.matmul(psd[:], dmyL[:], dmyR[:],
                                 start=True, stop=True)

            def fillers(b):
                # filler blocks (for batch b-1) usable while batch b's exps
                # pace the PE.  s3 blocks x: 0=(A,h0) 1=(B,h0) 2=(A,h1)
                # 3=(B,h1); order keeps A blocks early (need only F) and B
                # blocks after all g blocks (need Gn complete).  Consumed
                # via a carried deque (slots k=1..7 each batch) so no
                # leftover block ever sits between batches ahead of the
                # next batch's first z matmul.
                if b < 1:
                    return []
                fb = b - 1
                return [("g", fb, 0), ("g", fb, 1), ("g", fb, 2),
                        ("g", fb, 3), ("s3", fb, 0), ("s3", fb, 2),
                        ("s3", fb, 1), ("s3", fb, 3)]

            def run_filler(f, on_act=False):
                kind, fb, i = f
                if kind == "g":
                    _emit_g(nc, ts[fb], psG, i)
                else:
                    _emit_s3(nc, ts[fb], psAB, i, on_act=on_act)

            last = BPC - 1
            for b in range(BPC):
                t = ts[b]
                fl = fillers(b)
                for k in range(3):
                    _emit_z(nc, t, psS, rs_all, k)
                for k in range(3, NK):
                    if fl:
                        run_filler(fl.pop(0))
                    _emit_z(nc, t, psS, rs_all, k)
                    # last batch: quarter xbars so the drain's F turnaround
                    # after the final exp is ~1.3us instead of ~3us
                    if b == last:
                        if k == 3:
                            _emit_xbar(nc, t, 0, nsplit=4)
                        elif k == 5:
                            _emit_xbar(nc, t, 1, nsplit=4)
                        elif k == 7:
                            _emit_xbar(nc, t, 2, nsplit=4)
                    elif k == 3:
                        _emit_xbar(nc, t, 0)
                if b == last:
                    last_xbar = _emit_xbar(nc, t, 3, nsplit=4)
                else:
                    _emit_xbar(nc, t, 1)
                for f in fl:
                    run_filler(f)
                if b >= 1:
                    # reduce for the PREVIOUS batch: its E is long complete,
                    # so this never blocks the DVE queue head (emitting
                    # reduce(b) here would stall crec/Gn behind it waiting
                    # for exp(b,7), jamming psG recycling and the PE)
                    _emit_reduce(nc, ts[b - 1], rs_all)
            # all output stores AFTER the last xbar (no transpose/store
            # DMA-mode transitions mid-kernel), enforced with explicit
            # dependency edges on the final transpose
            for fb in range(BPC - 1):
                _emit_out(nc, ts[fb], after=last_xbar)
            # drain: g(3) + s3(3); drain copies go on ACT
            # (done with exps) instead of DVE
            for f in fillers(BPC):
                run_filler(f, on_act=True)
            _emit_reduce(nc, ts[last], rs_all)
            ins = nc.sync.dma_start(RS_d, rs_all[:])
            add_dep_helper(ins.ins, last_xbar.ins, reason="RS after last transpose")
            # last batch ships as A-half then B-half so the A rows go out
            # while the B drain copies still run
            t3 = ts[last]
            for isb in range(2):
                ins = nc.sync.dma_start(
                    t3.OUT_d[isb * D:(isb + 1) * D, :],
                    t3.AB[:, isb * 2 * QL:(isb + 1) * 2 * QL])
                add_dep_helper(ins.ins, last_xbar.ins,
                               reason="store after last transpose")
    nc.compile()
    return nc


def _get_nc():
    global _NC
    if _NC is None:
        _NC = _build()
    return _NC


def _prep_core(Ci, Qi, Wi):
    """Host-side layout prep for one core's shard (transposes/casts and
    cheap elementwise folds: UT = wq^T + wqc^T*C, rbias = sum_d wc*C^T)."""
    bpc = Ci.shape[0]
    pk = np.empty((bpc, D, PKW), dtype=BF)
    pk[:, :, O_QB:O_QB + QL] = Qi.astype(BF)
    qt = Qi.transpose(0, 2, 1).reshape(bpc, NJ, D, D).transpose(0, 2, 1, 3)
    pk[:, :, O_QT:O_QT + NJ * D] = qt.reshape(bpc, D, NJ * D).astype(BF)
    wqT = Wi[:, :, 0:D].transpose(0, 2, 1)          # (bpc, D, CL) f32
    wqcT = Wi[:, :, 2 * D:3 * D].transpose(0, 2, 1)
    pk[:, :, O_UT:O_UT + CL] = (wqT + wqcT * Ci).astype(BF)
    ct = Ci.transpose(0, 2, 1).reshape(bpc, NK, D, D).transpose(0, 2, 1, 3)
    cto = np.concatenate(
        [ct, np.ones((bpc, D, NK, 1), dtype=np.float32)], axis=3)
    pk[:, :, O_CTO:O_CTO + NK * (D + 1)] = (
        cto.reshape(bpc, D, NK * (D + 1)).astype(BF))
    # rbias[c] = sum_d wc[c,d] * C[d,c], laid out [c_local, k]
    rb = np.einsum('bcd,bdc->bc', Wi[:, :, D:2 * D], Ci).astype(np.float32)
    rb = rb.reshape(bpc, NK, D).transpose(0, 2, 1)  # (bpc, c_local, k)
    return {"PK": pk, "RB": np.ascontiguousarray(rb)}


def kernel(C, Q, W):
    C = np.ascontiguousarray(np.asarray(C, dtype=np.float32))
    Q = np.ascontiguousarray(np.asarray(Q, dtype=np.float32))
    W = np.ascontiguousarray(np.asarray(W, dtype=np.float32)).reshape(B, CL, 3 * D)
    in_maps = [
        _prep_core(C[i * BPC:(i + 1) * BPC],
                   Q[i * BPC:(i + 1) * BPC],
                   W[i * BPC:(i + 1) * BPC])
        for i in range(NCORES)
    ]
    nc = _get_nc()
    res = run_bass_kernel_spmd(nc, in_maps, core_ids=list(range(NCORES)), **RUN_KWARGS)
    global LAST_RESULT
    LAST_RESULT = res
    AB = np.concatenate([res.results[i]["OUT"] for i in range(NCORES)], axis=0)
    RS = np.stack([res.results[i]["RS"] for i in range(NCORES)])  # [8,D,BPC*NK]
    # host post: columnwise 1/rowsum (S1 normalization commuted out of the
    # A/B matmuls), then assemble [C, A, C*A, C*B]
    rowsum = (RS.reshape(NCORES, D, BPC, NK).transpose(0, 2, 3, 1)
              .reshape(B, CL))                       # [b, c] (k-major)
    h = (1.0 / rowsum)[:, None, :]                   # [b, 1, c]
    A = AB[:, 0:D].astype(np.float32) * h
    Bm = AB[:, D:2 * D].astype(np.float32) * h
    out = np.empty((B, 4 * D, CL), dtype=np.float32)
    out[:, 0:D] = C
    out[:, D:2 * D] = A
    out[:, 2 * D:3 * D] = C * A
    out[:, 3 * D:4 * D] = C * Bm
    return out
